# revision 8
# baseline (speedup 1.0000x reference)
"""GPTQ int4 quant linear: y = x @ dequant(qweight) + bias on 8 TRN2 cores.

Sharding: 2-way over tokens x 4-way over out_features (core c = (ti, oj)).
Each core: x shard [4096, 4096] (67 MB), weight shard [4096k, 1024n].

v2: the PE runs nothing but the 2048 N=512 matmuls (~437 us streaming
floor). The x transposes moved off the PE entirely:
  - x streams in as bf16 via GPSIMD cast-DMA (f32->bf16 in the DMA
    datapath), 256-token chunks laid out [128p, 2, 4096k],
  - the DMA xbar transposes each [256 tok, 128 k] slice SBUF->SBUF into
    xT tiles [128k, 256tok] (2-byte dtype path, fabric-rate),
  - weight dequant broadcasts qzeros/scales straight from DRAM
    ([0,16]-stride SWDGE APs) - no szp DRAM round-trip.
Dequantized W (bf16) stays resident: 32 tiles [128, 1024].
PSUM holds y [128,1024] f32 (2 banks), double-buffered; DVE adds bias
on eviction; y-out DMAs ride the scalar HWDGE queue, transposes the
sync queue, x-in the gpsimd queue.
"""

import numpy as np

import concourse.bass as bass
import concourse.mybir as mybir
import concourse.tile as tile
from concourse import bacc

F32 = mybir.dt.float32
I32 = mybir.dt.int32
BF16 = mybir.dt.bfloat16

N_CORES = 8
N_TOK_SHARDS = 2
N_OUT_SHARDS = 4
TOK = 8192
IN_F = 4096
OUT_F = 4096
TOK_SH = TOK // N_TOK_SHARDS  # 4096
OUT_SH = OUT_F // N_OUT_SHARDS  # 1024
PACKED_K = IN_F // 8  # 512 packed rows
GROUPSIZE = 128
N_GROUPS = IN_F // GROUPSIZE  # 32
P = 128
CHUNK = 256  # tokens per x chunk (2 m-tiles)
B = CHUNK // P  # 2

ALU = mybir.AluOpType


def build_nc(tok=TOK_SH):
    n_mtiles = tok // P  # 32
    n_chunks = tok // CHUNK  # 16
    n_t = PACKED_K // P  # 4 packed-row tiles
    n_kt = n_t * 8  # 32 k-tiles
    nc = bacc.Bacc(None, target_bir_lowering=False)

    x = nc.dram_tensor("x", [tok, IN_F], F32, kind="ExternalInput")
    qw = nc.dram_tensor("qw", [PACKED_K, OUT_SH], I32, kind="ExternalInput")
    qz = nc.dram_tensor("qz", [N_GROUPS, OUT_SH // 8], I32, kind="ExternalInput")
    sc = nc.dram_tensor("sc", [N_GROUPS, OUT_SH], F32, kind="ExternalInput")
    bi = nc.dram_tensor("bi", [1, OUT_SH], F32, kind="ExternalInput")
    out = nc.dram_tensor("out", [tok, OUT_SH], F32, kind="ExternalOutput")

    with tile.TileContext(nc) as tc:
        with (
            tc.tile_pool(name="singles", bufs=1) as singles,
            tc.tile_pool(name="weights", bufs=1) as wpool,
            tc.tile_pool(name="dq", bufs=2) as dqpool,
            tc.tile_pool(name="scexp", bufs=2) as scpool,
            tc.tile_pool(name="xin", bufs=2) as xpool,
            tc.tile_pool(name="xt", bufs=2) as xtpool,
            tc.tile_pool(name="yout", bufs=3) as ypool,
            tc.tile_pool(name="psum_y", bufs=2, space="PSUM") as psum_y,
        ):
            bias_sb = singles.tile([P, OUT_SH], F32)
            nc.gpsimd.dma_start(out=bias_sb, in_=bi[:, :].to_broadcast((P, OUT_SH)))

            # x chunk pipeline: cast-DMA f32->bf16 into [128, B, IN_F] with
            # token t = a + 128*b at (a, b, :), then per (k-tile, b) xbar
            # transpose [128 tok, 128 k] -> xT [128 k, 128 tok] SBUF->SBUF
            xts = {}

            def load_chunk(mc):
                xb = xpool.tile([P, B, IN_F], BF16, tag="xb")
                nc.gpsimd.dma_start(
                    xb,
                    x[mc * CHUNK : (mc + 1) * CHUNK, :].rearrange(
                        "(b a) k -> a b k", b=B
                    ),
                )
                for kt in range(n_kt):
                    for b in range(B):
                        xt = xtpool.tile([P, P], BF16, tag=f"xt{kt}b{b}")
                        nc.sync.dma_start_transpose(
                            xt, xb[:, b, kt * P : (kt + 1) * P]
                        )
                        xts[(mc, kt, b)] = xt

            load_chunk(0)

            # prefetch weight shard DMAs so dequant starts ASAP
            qw_tiles = []
            for t in range(n_t):
                qw_t = dqpool.tile([P, OUT_SH], I32, tag="qw")
                nc.sync.dma_start(qw_t, qw[t * P : (t + 1) * P, :])
                qw_tiles.append(qw_t)

            if n_chunks > 1:
                load_chunk(1)

            # ---- dequantize weight shard into 32 resident tiles ----
            # w[kk, n] = sc_exp[kk,n] * nib_j(qw)[kk,n] - sc_exp[kk,n]*(zq[kk,n]+1)
            w_tiles = []
            for t in range(n_t):
                # scale_exp[kk, n] = scales[8t + kk//16, n]
                scale_exp = scpool.tile([P, OUT_SH], F32, tag="scale_exp")
                nc.gpsimd.dma_start(
                    out=scale_exp,
                    in_=bass.AP(
                        tensor=sc,
                        offset=t * 8 * OUT_SH,
                        ap=[[OUT_SH, 8], [0, 16], [1, OUT_SH]],
                    ),
                )
                # qz_exp[kk, m] = qzeros[8t + kk//16, m]
                qz_exp = scpool.tile([P, OUT_SH // 8], I32, tag="qz_exp")
                nc.gpsimd.dma_start(
                    out=qz_exp,
                    in_=bass.AP(
                        tensor=qz,
                        offset=t * 8 * (OUT_SH // 8),
                        ap=[[OUT_SH // 8, 8], [0, 16], [1, OUT_SH // 8]],
                    ),
                )
                # szp_exp[kk, m*8+j] = scale_exp[kk, m*8+j] * (zq_nib_j[kk, m]+1)
                szp_exp = scpool.tile([P, OUT_SH], BF16, tag="szp_exp")
                szp_r = szp_exp.rearrange("p (m j) -> p m j", j=8)
                sc_r = scale_exp.rearrange("p (m j) -> p m j", j=8)
                zq_nib = scpool.tile([P, OUT_SH // 8], I32, tag="zq_nib")
                for j in range(8):
                    nc.vector.tensor_scalar(
                        out=zq_nib,
                        in0=qz_exp,
                        scalar1=4 * j,
                        scalar2=0xF,
                        op0=ALU.logical_shift_right,
                        op1=ALU.bitwise_and,
                    )
                    nc.vector.scalar_tensor_tensor(
                        out=szp_r[:, :, j],
                        in0=zq_nib,
                        scalar=1.0,
                        in1=sc_r[:, :, j],
                        op0=ALU.add,
                        op1=ALU.mult,
                    )
                qw_t = qw_tiles[t]
                for j in range(8):
                    kt = t * 8 + j
                    nib = dqpool.tile([P, OUT_SH], I32, tag="nib")
                    nc.vector.tensor_scalar(
                        out=nib,
                        in0=qw_t,
                        scalar1=4 * j,
                        scalar2=0xF,
                        op0=ALU.logical_shift_right,
                        op1=ALU.bitwise_and,
                    )
                    w = wpool.tile([P, OUT_SH], BF16, tag=f"w{kt}")
                    nc.vector.tensor_tensor(
                        out=w, in0=nib, in1=scale_exp, op=ALU.mult
                    )
                    nc.vector.tensor_sub(w, w, szp_exp)
                    w_tiles.append(w)

            # ---- main loop: chunks of 2 m-tiles, k-major accumulation ----
            for mc in range(n_chunks):
                if mc + 2 < n_chunks:
                    load_chunk(mc + 2)
                ypsums = {}
                for mi in range(B):
                    yp = psum_y.tile([P, OUT_SH], F32, tag="y")
                    ypsums[mi] = yp
                for kt in range(n_kt):
                    for mi in range(B):
                        lhsT = xts[(mc, kt, mi)]
                        for h in range(2):
                            nc.tensor.matmul(
                                ypsums[mi][:, h * 512 : (h + 1) * 512],
                                lhsT=lhsT,
                                rhs=w_tiles[kt][:, h * 512 : (h + 1) * 512],
                                start=(kt == 0),
                                stop=(kt == n_kt - 1),
                            )
                for mi in range(B):
                    y_sb = ypool.tile([P, OUT_SH], F32, tag="y_sb")
                    nc.vector.tensor_add(y_sb, ypsums[mi], bias_sb)
                    nc.scalar.dma_start(
                        out[mc * CHUNK + mi * P : mc * CHUNK + (mi + 1) * P, :],
                        y_sb,
                    )
                for key in [k for k in xts if k[0] == mc]:
                    del xts[key]

    nc.compile()
    return nc


_NC_CACHE = {}


def _get_nc(tok=TOK_SH):
    if tok not in _NC_CACHE:
        _NC_CACHE[tok] = build_nc(tok)
    return _NC_CACHE[tok]


# Device k-tile kt = t*8 + j holds W rows k = 8*(t*128+i) + j (nibble-major
# unpack order). Permute x columns so contiguous 128-col block kt matches.
_C = np.arange(IN_F)
_KT, _I = divmod(_C, P)
_T, _J = divmod(_KT, 8)
K_PERM = _T * 1024 + 8 * _I + _J


def _shard_inputs(x, qweight, qzeros, scales, bias, tok_sh=TOK_SH):
    xp = np.ascontiguousarray(x[:, K_PERM], dtype=np.float32)
    in_maps = []
    for c in range(N_CORES):
        ti, oj = divmod(c, N_OUT_SHARDS)
        sl = slice(oj * OUT_SH, (oj + 1) * OUT_SH)
        slz = slice(oj * (OUT_SH // 8), (oj + 1) * (OUT_SH // 8))
        in_maps.append(
            {
                "x": np.ascontiguousarray(
                    xp[ti * tok_sh : (ti + 1) * tok_sh], dtype=np.float32
                ),
                "qw": np.ascontiguousarray(qweight[:, sl], dtype=np.int32),
                "qz": np.ascontiguousarray(qzeros[:, slz], dtype=np.int32),
                "sc": np.ascontiguousarray(scales[:, sl], dtype=np.float32),
                "bi": np.ascontiguousarray(
                    bias[sl].reshape(1, OUT_SH), dtype=np.float32
                ),
            }
        )
    return in_maps


def _assemble(per_core, tok_sh=TOK_SH):
    out = np.empty((N_TOK_SHARDS * tok_sh, OUT_F), dtype=np.float32)
    for c in range(N_CORES):
        ti, oj = divmod(c, N_OUT_SHARDS)
        out[ti * tok_sh : (ti + 1) * tok_sh, oj * OUT_SH : (oj + 1) * OUT_SH] = (
            per_core[c]["out"]
        )
    return out


class PjrtRunner:
    """Builds the shard_map'd bass executable once; supports timed re-runs."""

    def __init__(self, nc):
        import jax
        from jax.sharding import Mesh, PartitionSpec
        from jax.experimental.shard_map import shard_map
        from concourse import bass2jax, mybir as mb

        self.jax = jax
        bass2jax.install_neuronx_cc_hook()

        partition_name = (
            nc.partition_id_tensor.name if nc.partition_id_tensor else None
        )
        in_names, out_names, out_avals, zero_outs = [], [], [], []
        for alloc in nc.m.functions[0].allocations:
            if not isinstance(alloc, mb.MemoryLocationSet):
                continue
            name = alloc.memorylocations[0].name
            if alloc.kind == "ExternalInput":
                if name != partition_name:
                    in_names.append(name)
            elif alloc.kind == "ExternalOutput":
                shape = tuple(alloc.tensor_shape)
                dtype = mb.dt.np(alloc.dtype)
                out_names.append(name)
                out_avals.append(jax.core.ShapedArray(shape, dtype))
                zero_outs.append(np.zeros(shape, dtype))
        self.in_names = in_names
        self.out_names = out_names
        self.zero_outs = zero_outs
        n_params = len(in_names)
        all_in_names = in_names + out_names
        if partition_name is not None:
            all_in_names.append(partition_name)

        def _body(*args):
            operands = list(args)
            if partition_name is not None:
                operands.append(bass2jax.partition_id_tensor())
            outs = bass2jax._bass_exec_p.bind(
                *operands,
                out_avals=tuple(out_avals),
                in_names=tuple(all_in_names),
                out_names=tuple(out_names),
                lowering_input_output_aliases=(),
                sim_require_finite=True,
                sim_require_nnan=True,
                nc=nc,
            )
            return tuple(outs)

        devices = jax.devices()[:N_CORES]
        self.mesh = Mesh(np.asarray(devices), ("core",))
        in_specs = (PartitionSpec("core"),) * (n_params + len(out_names))
        out_specs = (PartitionSpec("core"),) * len(out_names)
        # no donation: lets us re-run with the same device-resident inputs
        self.fn = jax.jit(
            shard_map(
                _body,
                mesh=self.mesh,
                in_specs=in_specs,
                out_specs=out_specs,
                check_rep=False,
            ),
            keep_unused=True,
        )
        self.out_avals = out_avals

    def stage_inputs(self, in_maps):
        import jax
        from jax.sharding import NamedSharding, PartitionSpec

        sharding = NamedSharding(self.mesh, PartitionSpec("core"))
        args = []
        for name in self.in_names:
            concat = np.concatenate([np.asarray(m[name]) for m in in_maps], axis=0)
            args.append(jax.device_put(concat, sharding))
        for z in self.zero_outs:
            zc = np.zeros((N_CORES * z.shape[0], *z.shape[1:]), z.dtype)
            args.append(jax.device_put(zc, sharding))
        self.args = args

    def run(self):
        outs = self.fn(*self.args)
        self.jax.block_until_ready(outs)
        return outs

    def outputs_to_numpy(self, outs):
        per_core = []
        for c in range(N_CORES):
            per_core.append(
                {
                    name: np.asarray(outs[i]).reshape(
                        N_CORES, *self.out_avals[i].shape
                    )[c]
                    for i, name in enumerate(self.out_names)
                }
            )
        return per_core


_RUNNER_CACHE = {}


def get_runner(tok=TOK_SH):
    if tok not in _RUNNER_CACHE:
        _RUNNER_CACHE[tok] = PjrtRunner(_get_nc(tok))
    return _RUNNER_CACHE[tok]


def _kernel_np_fallback(x, qweight, qzeros, scales, g_idx, bias):
    shifts = (np.arange(8, dtype=np.int64) * 4)[None, :, None]
    wq = ((qweight.astype(np.int64)[:, None, :] >> shifts) & 0xF).reshape(
        IN_F, qweight.shape[1]
    )
    zq = (
        (qzeros.astype(np.int64)[:, :, None] >> shifts.reshape(1, 1, 8)) & 0xF
    ).reshape(qzeros.shape[0], -1) + 1
    w = scales[g_idx] * (wq.astype(np.float32) - zq[g_idx].astype(np.float32))
    return (x.astype(np.float32) @ w + bias).astype(np.float32)


def kernel(x, qweight, qzeros, scales, g_idx, bias):
    x = np.asarray(x)
    qweight = np.asarray(qweight)
    qzeros = np.asarray(qzeros)
    scales = np.asarray(scales)
    g_idx = np.asarray(g_idx)
    bias = np.asarray(bias)

    if not np.array_equal(
        g_idx, (np.arange(IN_F, dtype=np.int64) // GROUPSIZE).astype(g_idx.dtype)
    ):
        return _kernel_np_fallback(x, qweight, qzeros, scales, g_idx, bias)

    runner = get_runner()
    runner.stage_inputs(_shard_inputs(x, qweight, qzeros, scales, bias))
    outs = runner.run()
    return _assemble(runner.outputs_to_numpy(outs))


# revision 12
# speedup vs baseline: 1.8330x; 1.8330x over previous
"""GPTQ int4 quant linear: y = x @ dequant(qweight) + bias on 8 TRN2 cores.

Sharding: 2-way over tokens x 4-way over out_features (core c = (ti, oj)).
Each core: x shard [4096, 4096] (67 MB), weight shard [4096k, 1024n].

v2: the PE runs nothing but the 2048 N=512 matmuls (~437 us streaming
floor). The x transposes moved off the PE entirely:
  - x streams in as bf16 via GPSIMD cast-DMA (f32->bf16 in the DMA
    datapath), 256-token chunks laid out [128p, 2, 4096k],
  - the DMA xbar transposes each [256 tok, 128 k] slice SBUF->SBUF into
    xT tiles [128k, 256tok] (2-byte dtype path, fabric-rate),
  - weight dequant broadcasts qzeros/scales straight from DRAM
    ([0,16]-stride SWDGE APs) - no szp DRAM round-trip.
Dequantized W (bf16) stays resident: 32 tiles [128, 1024].
PSUM holds y [128,1024] f32 (2 banks), double-buffered; DVE adds bias
on eviction; y-out DMAs ride the scalar HWDGE queue, transposes the
sync queue, x-in the gpsimd queue.
"""

import numpy as np

import concourse.bass as bass
import concourse.mybir as mybir
import concourse.tile as tile
from concourse import bacc

F32 = mybir.dt.float32
I32 = mybir.dt.int32
BF16 = mybir.dt.bfloat16

N_CORES = 8
N_TOK_SHARDS = 2
N_OUT_SHARDS = 4
TOK = 8192
IN_F = 4096
OUT_F = 4096
TOK_SH = TOK // N_TOK_SHARDS  # 4096
OUT_SH = OUT_F // N_OUT_SHARDS  # 1024
PACKED_K = IN_F // 8  # 512 packed rows
GROUPSIZE = 128
N_GROUPS = IN_F // GROUPSIZE  # 32
P = 128
CHUNK = 512  # tokens per x chunk (4 m-tiles)
B = CHUNK // P  # 4

ALU = mybir.AluOpType


def build_nc(tok=TOK_SH):
    n_mtiles = tok // P  # 32
    n_chunks = tok // CHUNK  # 16
    n_t = PACKED_K // P  # 4 packed-row tiles
    n_kt = n_t * 8  # 32 k-tiles
    nc = bacc.Bacc(None, target_bir_lowering=False)

    x = nc.dram_tensor("x", [tok, IN_F], F32, kind="ExternalInput")
    qw = nc.dram_tensor("qw", [PACKED_K, OUT_SH], I32, kind="ExternalInput")
    qz = nc.dram_tensor("qz", [N_GROUPS, OUT_SH // 8], I32, kind="ExternalInput")
    sc = nc.dram_tensor("sc", [N_GROUPS, OUT_SH], F32, kind="ExternalInput")
    bi = nc.dram_tensor("bi", [1, OUT_SH], F32, kind="ExternalInput")
    out = nc.dram_tensor("out", [tok, OUT_SH], F32, kind="ExternalOutput")

    with tile.TileContext(nc) as tc:
        with (
            tc.tile_pool(name="singles", bufs=1) as singles,
            tc.tile_pool(name="weights", bufs=1) as wpool,
            tc.tile_pool(name="dq", bufs=2) as dqpool,
            tc.tile_pool(name="scexp", bufs=2) as scpool,
            tc.tile_pool(name="xt", bufs=2) as xtpool,
            tc.tile_pool(name="yout", bufs=3) as ypool,
            tc.tile_pool(name="psum_y", bufs=2, space="PSUM") as psum_y,
            tc.tile_pool(name="dram", bufs=1, space="DRAM") as drampool,
        ):
            bias_sb = singles.tile([P, OUT_SH], F32)
            nc.gpsimd.dma_start(out=bias_sb, in_=bi[:, :].to_broadcast((P, OUT_SH)))

            # x pipeline: per chunk, SWDGE cast-DMA f32->bf16 DRAM->DRAM,
            # then per k-tile a big xbar transpose DRAM->SBUF:
            # [CHUNK tok, 128 k] -> xT [128 k, CHUNK tok]
            x16 = drampool.tile([tok, IN_F], BF16)
            xts = {}

            def cast_chunk(mc):
                nc.gpsimd.dma_start(
                    x16[mc * CHUNK : (mc + 1) * CHUNK, :],
                    x[mc * CHUNK : (mc + 1) * CHUNK, :],
                )

            def load_chunk(mc):
                rows = x16[mc * CHUNK : (mc + 1) * CHUNK, :]
                for kt in range(n_kt):
                    xt = xtpool.tile([P, CHUNK], BF16, tag=f"xt{kt}")
                    nc.sync.dma_start_transpose(
                        xt, rows[:, kt * P : (kt + 1) * P]
                    )
                    xts[(mc, kt)] = xt

            cast_chunk(0)
            load_chunk(0)

            # prefetch weight shard DMAs so dequant starts ASAP
            qw_tiles = []
            for t in range(n_t):
                qw_t = dqpool.tile([P, OUT_SH], I32, tag="qw")
                nc.scalar.dma_start(qw_t, qw[t * P : (t + 1) * P, :])
                qw_tiles.append(qw_t)

            if n_chunks > 1:
                cast_chunk(1)
                load_chunk(1)

            # ---- dequantize weight shard into 32 resident tiles ----
            # w[kk, n] = sc_exp[kk,n] * nib_j(qw)[kk,n] - sc_exp[kk,n]*(zq[kk,n]+1)
            w_tiles = []
            for t in range(n_t):
                # scale_exp[kk, n] = scales[8t + kk//16, n]
                scale_exp = scpool.tile([P, OUT_SH], F32, tag="scale_exp")
                nc.gpsimd.dma_start(
                    out=scale_exp,
                    in_=bass.AP(
                        tensor=sc,
                        offset=t * 8 * OUT_SH,
                        ap=[[OUT_SH, 8], [0, 16], [1, OUT_SH]],
                    ),
                )
                # qz_exp[kk, m] = qzeros[8t + kk//16, m]
                qz_exp = scpool.tile([P, OUT_SH // 8], I32, tag="qz_exp")
                nc.gpsimd.dma_start(
                    out=qz_exp,
                    in_=bass.AP(
                        tensor=qz,
                        offset=t * 8 * (OUT_SH // 8),
                        ap=[[OUT_SH // 8, 8], [0, 16], [1, OUT_SH // 8]],
                    ),
                )
                # szp_exp[kk, m*8+j] = scale_exp[kk, m*8+j] * (zq_nib_j[kk, m]+1)
                szp_exp = scpool.tile([P, OUT_SH], BF16, tag="szp_exp")
                szp_r = szp_exp.rearrange("p (m j) -> p m j", j=8)
                sc_r = scale_exp.rearrange("p (m j) -> p m j", j=8)
                zq_nib = scpool.tile([P, OUT_SH // 8], I32, tag="zq_nib")
                for j in range(8):
                    nc.vector.tensor_scalar(
                        out=zq_nib,
                        in0=qz_exp,
                        scalar1=4 * j,
                        scalar2=0xF,
                        op0=ALU.logical_shift_right,
                        op1=ALU.bitwise_and,
                    )
                    nc.vector.scalar_tensor_tensor(
                        out=szp_r[:, :, j],
                        in0=zq_nib,
                        scalar=1.0,
                        in1=sc_r[:, :, j],
                        op0=ALU.add,
                        op1=ALU.mult,
                    )
                qw_t = qw_tiles[t]
                for j in range(8):
                    kt = t * 8 + j
                    nib = dqpool.tile([P, OUT_SH], I32, tag="nib")
                    nc.vector.tensor_scalar(
                        out=nib,
                        in0=qw_t,
                        scalar1=4 * j,
                        scalar2=0xF,
                        op0=ALU.logical_shift_right,
                        op1=ALU.bitwise_and,
                    )
                    w = wpool.tile([P, OUT_SH], BF16, tag=f"w{kt}")
                    nc.vector.tensor_tensor(
                        out=w, in0=nib, in1=scale_exp, op=ALU.mult
                    )
                    nc.vector.tensor_sub(w, w, szp_exp)
                    w_tiles.append(w)

            # ---- main loop: chunks of 4 m-tiles, processed in halves of 2
            # (2 y-psum tiles x 2 pool bufs = all 8 PSUM banks) ----
            for mc in range(n_chunks):
                if mc + 2 < n_chunks:
                    cast_chunk(mc + 2)
                    load_chunk(mc + 2)
                for half in range(B // 2):
                    mis = (2 * half, 2 * half + 1)
                    ypsums = {}
                    for mi in mis:
                        yp = psum_y.tile([P, OUT_SH], F32, tag="y")
                        ypsums[mi] = yp
                    for kt in range(n_kt):
                        for mi in mis:
                            lhsT = xts[(mc, kt)][:, mi * P : (mi + 1) * P]
                            for h in range(2):
                                nc.tensor.matmul(
                                    ypsums[mi][:, h * 512 : (h + 1) * 512],
                                    lhsT=lhsT,
                                    rhs=w_tiles[kt][:, h * 512 : (h + 1) * 512],
                                    start=(kt == 0),
                                    stop=(kt == n_kt - 1),
                                )
                    for mi in mis:
                        y_sb = ypool.tile([P, OUT_SH], F32, tag="y_sb")
                        nc.vector.tensor_add(y_sb, ypsums[mi], bias_sb)
                        nc.scalar.dma_start(
                            out[
                                mc * CHUNK + mi * P : mc * CHUNK + (mi + 1) * P, :
                            ],
                            y_sb,
                        )
                for key in [k for k in xts if k[0] == mc]:
                    del xts[key]

    nc.compile()
    return nc


_NC_CACHE = {}


def _get_nc(tok=TOK_SH):
    if tok not in _NC_CACHE:
        _NC_CACHE[tok] = build_nc(tok)
    return _NC_CACHE[tok]


# Device k-tile kt = t*8 + j holds W rows k = 8*(t*128+i) + j (nibble-major
# unpack order). Permute x columns so contiguous 128-col block kt matches.
_C = np.arange(IN_F)
_KT, _I = divmod(_C, P)
_T, _J = divmod(_KT, 8)
K_PERM = _T * 1024 + 8 * _I + _J


def _shard_inputs(x, qweight, qzeros, scales, bias, tok_sh=TOK_SH):
    xp = np.ascontiguousarray(x[:, K_PERM], dtype=np.float32)
    in_maps = []
    for c in range(N_CORES):
        ti, oj = divmod(c, N_OUT_SHARDS)
        sl = slice(oj * OUT_SH, (oj + 1) * OUT_SH)
        slz = slice(oj * (OUT_SH // 8), (oj + 1) * (OUT_SH // 8))
        in_maps.append(
            {
                "x": np.ascontiguousarray(
                    xp[ti * tok_sh : (ti + 1) * tok_sh], dtype=np.float32
                ),
                "qw": np.ascontiguousarray(qweight[:, sl], dtype=np.int32),
                "qz": np.ascontiguousarray(qzeros[:, slz], dtype=np.int32),
                "sc": np.ascontiguousarray(scales[:, sl], dtype=np.float32),
                "bi": np.ascontiguousarray(
                    bias[sl].reshape(1, OUT_SH), dtype=np.float32
                ),
            }
        )
    return in_maps


def _assemble(per_core, tok_sh=TOK_SH):
    out = np.empty((N_TOK_SHARDS * tok_sh, OUT_F), dtype=np.float32)
    for c in range(N_CORES):
        ti, oj = divmod(c, N_OUT_SHARDS)
        out[ti * tok_sh : (ti + 1) * tok_sh, oj * OUT_SH : (oj + 1) * OUT_SH] = (
            per_core[c]["out"]
        )
    return out


class PjrtRunner:
    """Builds the shard_map'd bass executable once; supports timed re-runs."""

    def __init__(self, nc):
        import jax
        from jax.sharding import Mesh, PartitionSpec
        from jax.experimental.shard_map import shard_map
        from concourse import bass2jax, mybir as mb

        self.jax = jax
        bass2jax.install_neuronx_cc_hook()

        partition_name = (
            nc.partition_id_tensor.name if nc.partition_id_tensor else None
        )
        in_names, out_names, out_avals, zero_outs = [], [], [], []
        for alloc in nc.m.functions[0].allocations:
            if not isinstance(alloc, mb.MemoryLocationSet):
                continue
            name = alloc.memorylocations[0].name
            if alloc.kind == "ExternalInput":
                if name != partition_name:
                    in_names.append(name)
            elif alloc.kind == "ExternalOutput":
                shape = tuple(alloc.tensor_shape)
                dtype = mb.dt.np(alloc.dtype)
                out_names.append(name)
                out_avals.append(jax.core.ShapedArray(shape, dtype))
                zero_outs.append(np.zeros(shape, dtype))
        self.in_names = in_names
        self.out_names = out_names
        self.zero_outs = zero_outs
        n_params = len(in_names)
        all_in_names = in_names + out_names
        if partition_name is not None:
            all_in_names.append(partition_name)

        def _body(*args):
            operands = list(args)
            if partition_name is not None:
                operands.append(bass2jax.partition_id_tensor())
            outs = bass2jax._bass_exec_p.bind(
                *operands,
                out_avals=tuple(out_avals),
                in_names=tuple(all_in_names),
                out_names=tuple(out_names),
                lowering_input_output_aliases=(),
                sim_require_finite=True,
                sim_require_nnan=True,
                nc=nc,
            )
            return tuple(outs)

        devices = jax.devices()[:N_CORES]
        self.mesh = Mesh(np.asarray(devices), ("core",))
        in_specs = (PartitionSpec("core"),) * (n_params + len(out_names))
        out_specs = (PartitionSpec("core"),) * len(out_names)
        # no donation: lets us re-run with the same device-resident inputs
        self.fn = jax.jit(
            shard_map(
                _body,
                mesh=self.mesh,
                in_specs=in_specs,
                out_specs=out_specs,
                check_rep=False,
            ),
            keep_unused=True,
        )
        self.out_avals = out_avals

    def stage_inputs(self, in_maps):
        import jax
        from jax.sharding import NamedSharding, PartitionSpec

        sharding = NamedSharding(self.mesh, PartitionSpec("core"))
        args = []
        for name in self.in_names:
            concat = np.concatenate([np.asarray(m[name]) for m in in_maps], axis=0)
            args.append(jax.device_put(concat, sharding))
        for z in self.zero_outs:
            zc = np.zeros((N_CORES * z.shape[0], *z.shape[1:]), z.dtype)
            args.append(jax.device_put(zc, sharding))
        self.args = args

    def run(self):
        outs = self.fn(*self.args)
        self.jax.block_until_ready(outs)
        return outs

    def outputs_to_numpy(self, outs):
        per_core = []
        for c in range(N_CORES):
            per_core.append(
                {
                    name: np.asarray(outs[i]).reshape(
                        N_CORES, *self.out_avals[i].shape
                    )[c]
                    for i, name in enumerate(self.out_names)
                }
            )
        return per_core


_RUNNER_CACHE = {}


def get_runner(tok=TOK_SH):
    if tok not in _RUNNER_CACHE:
        _RUNNER_CACHE[tok] = PjrtRunner(_get_nc(tok))
    return _RUNNER_CACHE[tok]


def _kernel_np_fallback(x, qweight, qzeros, scales, g_idx, bias):
    shifts = (np.arange(8, dtype=np.int64) * 4)[None, :, None]
    wq = ((qweight.astype(np.int64)[:, None, :] >> shifts) & 0xF).reshape(
        IN_F, qweight.shape[1]
    )
    zq = (
        (qzeros.astype(np.int64)[:, :, None] >> shifts.reshape(1, 1, 8)) & 0xF
    ).reshape(qzeros.shape[0], -1) + 1
    w = scales[g_idx] * (wq.astype(np.float32) - zq[g_idx].astype(np.float32))
    return (x.astype(np.float32) @ w + bias).astype(np.float32)


def kernel(x, qweight, qzeros, scales, g_idx, bias):
    x = np.asarray(x)
    qweight = np.asarray(qweight)
    qzeros = np.asarray(qzeros)
    scales = np.asarray(scales)
    g_idx = np.asarray(g_idx)
    bias = np.asarray(bias)

    if not np.array_equal(
        g_idx, (np.arange(IN_F, dtype=np.int64) // GROUPSIZE).astype(g_idx.dtype)
    ):
        return _kernel_np_fallback(x, qweight, qzeros, scales, g_idx, bias)

    runner = get_runner()
    runner.stage_inputs(_shard_inputs(x, qweight, qzeros, scales, bias))
    outs = runner.run()
    return _assemble(runner.outputs_to_numpy(outs))


# revision 14
# speedup vs baseline: 2.2052x; 1.2030x over previous
"""GPTQ int4 quant linear: y = x @ dequant(qweight) + bias on 8 TRN2 cores.

Sharding: 2-way over tokens x 4-way over out_features (core c = (ti, oj)).
Each core: x shard [4096, 4096] (67 MB), weight shard [4096k, 1024n].

v2: the PE runs nothing but the 2048 N=512 matmuls (~437 us streaming
floor). The x transposes moved off the PE entirely:
  - x streams in as bf16 via GPSIMD cast-DMA (f32->bf16 in the DMA
    datapath), 256-token chunks laid out [128p, 2, 4096k],
  - the DMA xbar transposes each [256 tok, 128 k] slice SBUF->SBUF into
    xT tiles [128k, 256tok] (2-byte dtype path, fabric-rate),
  - weight dequant broadcasts qzeros/scales straight from DRAM
    ([0,16]-stride SWDGE APs) - no szp DRAM round-trip.
Dequantized W (bf16) stays resident: 32 tiles [128, 1024].
PSUM holds y [128,1024] f32 (2 banks), double-buffered; DVE adds bias
on eviction; y-out DMAs ride the scalar HWDGE queue, transposes the
sync queue, x-in the gpsimd queue.
"""

import numpy as np

import concourse.bass as bass
import concourse.mybir as mybir
import concourse.tile as tile
from concourse import bacc

F32 = mybir.dt.float32
I32 = mybir.dt.int32
BF16 = mybir.dt.bfloat16

N_CORES = 8
N_TOK_SHARDS = 2
N_OUT_SHARDS = 4
TOK = 8192
IN_F = 4096
OUT_F = 4096
TOK_SH = TOK // N_TOK_SHARDS  # 4096
OUT_SH = OUT_F // N_OUT_SHARDS  # 1024
PACKED_K = IN_F // 8  # 512 packed rows
GROUPSIZE = 128
N_GROUPS = IN_F // GROUPSIZE  # 32
P = 128
CHUNK = 512  # tokens per x chunk (4 m-tiles)
B = CHUNK // P  # 4

ALU = mybir.AluOpType


def build_nc(tok=TOK_SH):
    n_mtiles = tok // P  # 32
    n_chunks = tok // CHUNK  # 16
    n_t = PACKED_K // P  # 4 packed-row tiles
    n_kt = n_t * 8  # 32 k-tiles
    nc = bacc.Bacc(None, target_bir_lowering=False)

    x = nc.dram_tensor("x", [tok, IN_F], F32, kind="ExternalInput")
    qw = nc.dram_tensor("qw", [PACKED_K, OUT_SH], I32, kind="ExternalInput")
    qz = nc.dram_tensor("qz", [N_GROUPS, OUT_SH // 8], I32, kind="ExternalInput")
    sc = nc.dram_tensor("sc", [N_GROUPS, OUT_SH], F32, kind="ExternalInput")
    bi = nc.dram_tensor("bi", [1, OUT_SH], F32, kind="ExternalInput")
    out = nc.dram_tensor("out", [tok, OUT_SH], F32, kind="ExternalOutput")

    with tile.TileContext(nc) as tc:
        with (
            tc.tile_pool(name="singles", bufs=1) as singles,
            tc.tile_pool(name="weights", bufs=1) as wpool,
            tc.tile_pool(name="dq", bufs=2) as dqpool,
            tc.tile_pool(name="scexp", bufs=2) as scpool,
            tc.tile_pool(name="xt", bufs=2) as xtpool,
            tc.tile_pool(name="yout", bufs=3) as ypool,
            tc.tile_pool(name="psum_y", bufs=2, space="PSUM") as psum_y,
            tc.tile_pool(name="dram", bufs=1, space="DRAM") as drampool,
        ):
            bias_sb = singles.tile([P, OUT_SH], F32)
            nc.gpsimd.dma_start(out=bias_sb, in_=bi[:, :].to_broadcast((P, OUT_SH)))

            # x pipeline: per chunk, SWDGE cast-DMA f32->bf16 DRAM->DRAM,
            # then per k-tile a big xbar transpose DRAM->SBUF:
            # [CHUNK tok, 128 k] -> xT [128 k, CHUNK tok]
            # One DRAM tile per chunk: a single shared tile would make Tile
            # serialize each cast against every transpose reading it.
            x16 = {}
            xts = {}

            def cast_chunk(mc):
                x16_mc = drampool.tile([CHUNK, IN_F], BF16, tag=f"x16_{mc}")
                x16[mc] = x16_mc
                nc.gpsimd.dma_start(
                    x16_mc[:, :],
                    x[mc * CHUNK : (mc + 1) * CHUNK, :],
                )

            def load_chunk(mc):
                rows = x16[mc]
                for kt in range(n_kt):
                    xt = xtpool.tile([P, CHUNK], BF16, tag=f"xt{kt}")
                    nc.sync.dma_start_transpose(
                        xt, rows[:, kt * P : (kt + 1) * P]
                    )
                    xts[(mc, kt)] = xt

            cast_chunk(0)
            load_chunk(0)

            # prefetch weight shard DMAs so dequant starts ASAP
            qw_tiles = []
            for t in range(n_t):
                qw_t = dqpool.tile([P, OUT_SH], I32, tag="qw")
                nc.scalar.dma_start(qw_t, qw[t * P : (t + 1) * P, :])
                qw_tiles.append(qw_t)

            if n_chunks > 1:
                cast_chunk(1)
                load_chunk(1)

            # ---- dequantize weight shard into 32 resident tiles ----
            # w[kk, n] = sc_exp[kk,n] * nib_j(qw)[kk,n] - sc_exp[kk,n]*(zq[kk,n]+1)
            w_tiles = []
            for t in range(n_t):
                # scale_exp[kk, n] = scales[8t + kk//16, n]
                scale_exp = scpool.tile([P, OUT_SH], F32, tag="scale_exp")
                nc.gpsimd.dma_start(
                    out=scale_exp,
                    in_=bass.AP(
                        tensor=sc,
                        offset=t * 8 * OUT_SH,
                        ap=[[OUT_SH, 8], [0, 16], [1, OUT_SH]],
                    ),
                )
                # qz_exp[kk, m] = qzeros[8t + kk//16, m]
                qz_exp = scpool.tile([P, OUT_SH // 8], I32, tag="qz_exp")
                nc.gpsimd.dma_start(
                    out=qz_exp,
                    in_=bass.AP(
                        tensor=qz,
                        offset=t * 8 * (OUT_SH // 8),
                        ap=[[OUT_SH // 8, 8], [0, 16], [1, OUT_SH // 8]],
                    ),
                )
                # szp_exp[kk, m*8+j] = scale_exp[kk, m*8+j] * (zq_nib_j[kk, m]+1)
                szp_exp = scpool.tile([P, OUT_SH], BF16, tag="szp_exp")
                szp_r = szp_exp.rearrange("p (m j) -> p m j", j=8)
                sc_r = scale_exp.rearrange("p (m j) -> p m j", j=8)
                zq_nib = scpool.tile([P, OUT_SH // 8], I32, tag="zq_nib")
                for j in range(8):
                    nc.vector.tensor_scalar(
                        out=zq_nib,
                        in0=qz_exp,
                        scalar1=4 * j,
                        scalar2=0xF,
                        op0=ALU.logical_shift_right,
                        op1=ALU.bitwise_and,
                    )
                    nc.vector.scalar_tensor_tensor(
                        out=szp_r[:, :, j],
                        in0=zq_nib,
                        scalar=1.0,
                        in1=sc_r[:, :, j],
                        op0=ALU.add,
                        op1=ALU.mult,
                    )
                qw_t = qw_tiles[t]
                for j in range(8):
                    kt = t * 8 + j
                    nib = dqpool.tile([P, OUT_SH], I32, tag="nib")
                    nc.vector.tensor_scalar(
                        out=nib,
                        in0=qw_t,
                        scalar1=4 * j,
                        scalar2=0xF,
                        op0=ALU.logical_shift_right,
                        op1=ALU.bitwise_and,
                    )
                    w = wpool.tile([P, OUT_SH], BF16, tag=f"w{kt}")
                    nc.vector.tensor_tensor(
                        out=w, in0=nib, in1=scale_exp, op=ALU.mult
                    )
                    nc.vector.tensor_sub(w, w, szp_exp)
                    w_tiles.append(w)

            # ---- main loop: chunks of 4 m-tiles, processed in halves of 2
            # (2 y-psum tiles x 2 pool bufs = all 8 PSUM banks) ----
            for mc in range(n_chunks):
                if mc + 2 < n_chunks:
                    cast_chunk(mc + 2)
                    load_chunk(mc + 2)
                for half in range(B // 2):
                    mis = (2 * half, 2 * half + 1)
                    ypsums = {}
                    for mi in mis:
                        yp = psum_y.tile([P, OUT_SH], F32, tag="y")
                        ypsums[mi] = yp
                    for kt in range(n_kt):
                        for mi in mis:
                            lhsT = xts[(mc, kt)][:, mi * P : (mi + 1) * P]
                            for h in range(2):
                                nc.tensor.matmul(
                                    ypsums[mi][:, h * 512 : (h + 1) * 512],
                                    lhsT=lhsT,
                                    rhs=w_tiles[kt][:, h * 512 : (h + 1) * 512],
                                    start=(kt == 0),
                                    stop=(kt == n_kt - 1),
                                )
                    for mi in mis:
                        y_sb = ypool.tile([P, OUT_SH], F32, tag="y_sb")
                        nc.vector.tensor_add(y_sb, ypsums[mi], bias_sb)
                        nc.scalar.dma_start(
                            out[
                                mc * CHUNK + mi * P : mc * CHUNK + (mi + 1) * P, :
                            ],
                            y_sb,
                        )
                for key in [k for k in xts if k[0] == mc]:
                    del xts[key]

    nc.compile()
    return nc


_NC_CACHE = {}


def _get_nc(tok=TOK_SH):
    if tok not in _NC_CACHE:
        _NC_CACHE[tok] = build_nc(tok)
    return _NC_CACHE[tok]


# Device k-tile kt = t*8 + j holds W rows k = 8*(t*128+i) + j (nibble-major
# unpack order). Permute x columns so contiguous 128-col block kt matches.
_C = np.arange(IN_F)
_KT, _I = divmod(_C, P)
_T, _J = divmod(_KT, 8)
K_PERM = _T * 1024 + 8 * _I + _J


def _shard_inputs(x, qweight, qzeros, scales, bias, tok_sh=TOK_SH):
    xp = np.ascontiguousarray(x[:, K_PERM], dtype=np.float32)
    in_maps = []
    for c in range(N_CORES):
        ti, oj = divmod(c, N_OUT_SHARDS)
        sl = slice(oj * OUT_SH, (oj + 1) * OUT_SH)
        slz = slice(oj * (OUT_SH // 8), (oj + 1) * (OUT_SH // 8))
        in_maps.append(
            {
                "x": np.ascontiguousarray(
                    xp[ti * tok_sh : (ti + 1) * tok_sh], dtype=np.float32
                ),
                "qw": np.ascontiguousarray(qweight[:, sl], dtype=np.int32),
                "qz": np.ascontiguousarray(qzeros[:, slz], dtype=np.int32),
                "sc": np.ascontiguousarray(scales[:, sl], dtype=np.float32),
                "bi": np.ascontiguousarray(
                    bias[sl].reshape(1, OUT_SH), dtype=np.float32
                ),
            }
        )
    return in_maps


def _assemble(per_core, tok_sh=TOK_SH):
    out = np.empty((N_TOK_SHARDS * tok_sh, OUT_F), dtype=np.float32)
    for c in range(N_CORES):
        ti, oj = divmod(c, N_OUT_SHARDS)
        out[ti * tok_sh : (ti + 1) * tok_sh, oj * OUT_SH : (oj + 1) * OUT_SH] = (
            per_core[c]["out"]
        )
    return out


class PjrtRunner:
    """Builds the shard_map'd bass executable once; supports timed re-runs."""

    def __init__(self, nc):
        import jax
        from jax.sharding import Mesh, PartitionSpec
        from jax.experimental.shard_map import shard_map
        from concourse import bass2jax, mybir as mb

        self.jax = jax
        bass2jax.install_neuronx_cc_hook()

        partition_name = (
            nc.partition_id_tensor.name if nc.partition_id_tensor else None
        )
        in_names, out_names, out_avals, zero_outs = [], [], [], []
        for alloc in nc.m.functions[0].allocations:
            if not isinstance(alloc, mb.MemoryLocationSet):
                continue
            name = alloc.memorylocations[0].name
            if alloc.kind == "ExternalInput":
                if name != partition_name:
                    in_names.append(name)
            elif alloc.kind == "ExternalOutput":
                shape = tuple(alloc.tensor_shape)
                dtype = mb.dt.np(alloc.dtype)
                out_names.append(name)
                out_avals.append(jax.core.ShapedArray(shape, dtype))
                zero_outs.append(np.zeros(shape, dtype))
        self.in_names = in_names
        self.out_names = out_names
        self.zero_outs = zero_outs
        n_params = len(in_names)
        all_in_names = in_names + out_names
        if partition_name is not None:
            all_in_names.append(partition_name)

        def _body(*args):
            operands = list(args)
            if partition_name is not None:
                operands.append(bass2jax.partition_id_tensor())
            outs = bass2jax._bass_exec_p.bind(
                *operands,
                out_avals=tuple(out_avals),
                in_names=tuple(all_in_names),
                out_names=tuple(out_names),
                lowering_input_output_aliases=(),
                sim_require_finite=True,
                sim_require_nnan=True,
                nc=nc,
            )
            return tuple(outs)

        devices = jax.devices()[:N_CORES]
        self.mesh = Mesh(np.asarray(devices), ("core",))
        in_specs = (PartitionSpec("core"),) * (n_params + len(out_names))
        out_specs = (PartitionSpec("core"),) * len(out_names)
        # no donation: lets us re-run with the same device-resident inputs
        self.fn = jax.jit(
            shard_map(
                _body,
                mesh=self.mesh,
                in_specs=in_specs,
                out_specs=out_specs,
                check_rep=False,
            ),
            keep_unused=True,
        )
        self.out_avals = out_avals

    def stage_inputs(self, in_maps):
        import jax
        from jax.sharding import NamedSharding, PartitionSpec

        sharding = NamedSharding(self.mesh, PartitionSpec("core"))
        args = []
        for name in self.in_names:
            concat = np.concatenate([np.asarray(m[name]) for m in in_maps], axis=0)
            args.append(jax.device_put(concat, sharding))
        for z in self.zero_outs:
            zc = np.zeros((N_CORES * z.shape[0], *z.shape[1:]), z.dtype)
            args.append(jax.device_put(zc, sharding))
        self.args = args

    def run(self):
        outs = self.fn(*self.args)
        self.jax.block_until_ready(outs)
        return outs

    def outputs_to_numpy(self, outs):
        per_core = []
        for c in range(N_CORES):
            per_core.append(
                {
                    name: np.asarray(outs[i]).reshape(
                        N_CORES, *self.out_avals[i].shape
                    )[c]
                    for i, name in enumerate(self.out_names)
                }
            )
        return per_core


_RUNNER_CACHE = {}


def get_runner(tok=TOK_SH):
    if tok not in _RUNNER_CACHE:
        _RUNNER_CACHE[tok] = PjrtRunner(_get_nc(tok))
    return _RUNNER_CACHE[tok]


def _kernel_np_fallback(x, qweight, qzeros, scales, g_idx, bias):
    shifts = (np.arange(8, dtype=np.int64) * 4)[None, :, None]
    wq = ((qweight.astype(np.int64)[:, None, :] >> shifts) & 0xF).reshape(
        IN_F, qweight.shape[1]
    )
    zq = (
        (qzeros.astype(np.int64)[:, :, None] >> shifts.reshape(1, 1, 8)) & 0xF
    ).reshape(qzeros.shape[0], -1) + 1
    w = scales[g_idx] * (wq.astype(np.float32) - zq[g_idx].astype(np.float32))
    return (x.astype(np.float32) @ w + bias).astype(np.float32)


def kernel(x, qweight, qzeros, scales, g_idx, bias):
    x = np.asarray(x)
    qweight = np.asarray(qweight)
    qzeros = np.asarray(qzeros)
    scales = np.asarray(scales)
    g_idx = np.asarray(g_idx)
    bias = np.asarray(bias)

    if not np.array_equal(
        g_idx, (np.arange(IN_F, dtype=np.int64) // GROUPSIZE).astype(g_idx.dtype)
    ):
        return _kernel_np_fallback(x, qweight, qzeros, scales, g_idx, bias)

    runner = get_runner()
    runner.stage_inputs(_shard_inputs(x, qweight, qzeros, scales, bias))
    outs = runner.run()
    return _assemble(runner.outputs_to_numpy(outs))


# revision 17
# speedup vs baseline: 2.8487x; 1.2918x over previous
"""GPTQ int4 quant linear: y = x @ dequant(qweight) + bias on 8 TRN2 cores.

Sharding: 2-way over tokens x 4-way over out_features (core c = (ti, oj)).
Each core: x shard [4096, 4096] (67 MB), weight shard [4096k, 1024n].

v2: the PE runs nothing but the 2048 N=512 matmuls (~437 us streaming
floor). The x transposes moved off the PE entirely:
  - x streams in as bf16 via GPSIMD cast-DMA (f32->bf16 in the DMA
    datapath), 256-token chunks laid out [128p, 2, 4096k],
  - the DMA xbar transposes each [256 tok, 128 k] slice SBUF->SBUF into
    xT tiles [128k, 256tok] (2-byte dtype path, fabric-rate),
  - weight dequant broadcasts qzeros/scales straight from DRAM
    ([0,16]-stride SWDGE APs) - no szp DRAM round-trip.
Dequantized W (bf16) stays resident: 32 tiles [128, 1024].
PSUM holds y [128,1024] f32 (2 banks), double-buffered; DVE adds bias
on eviction; y-out DMAs ride the scalar HWDGE queue, transposes the
sync queue, x-in the gpsimd queue.
"""

import numpy as np

import concourse.bass as bass
import concourse.mybir as mybir
import concourse.tile as tile
from concourse import bacc

F32 = mybir.dt.float32
I32 = mybir.dt.int32
BF16 = mybir.dt.bfloat16

N_CORES = 8
N_TOK_SHARDS = 2
N_OUT_SHARDS = 4
TOK = 8192
IN_F = 4096
OUT_F = 4096
TOK_SH = TOK // N_TOK_SHARDS  # 4096
OUT_SH = OUT_F // N_OUT_SHARDS  # 1024
PACKED_K = IN_F // 8  # 512 packed rows
GROUPSIZE = 128
N_GROUPS = IN_F // GROUPSIZE  # 32
P = 128
CHUNK = 512  # tokens per x chunk (4 m-tiles)
B = CHUNK // P  # 4

ALU = mybir.AluOpType


def build_nc(tok=TOK_SH):
    n_mtiles = tok // P  # 32
    n_chunks = tok // CHUNK  # 16
    n_t = PACKED_K // P  # 4 packed-row tiles
    n_kt = n_t * 8  # 32 k-tiles
    nc = bacc.Bacc(None, target_bir_lowering=False)

    x = nc.dram_tensor("x", [tok, IN_F], BF16, kind="ExternalInput")
    qw = nc.dram_tensor("qw", [PACKED_K, OUT_SH], I32, kind="ExternalInput")
    qz = nc.dram_tensor("qz", [N_GROUPS, OUT_SH // 8], I32, kind="ExternalInput")
    sc = nc.dram_tensor("sc", [N_GROUPS, OUT_SH], F32, kind="ExternalInput")
    bi = nc.dram_tensor("bi", [1, OUT_SH], F32, kind="ExternalInput")
    out = nc.dram_tensor("out", [tok, OUT_SH], F32, kind="ExternalOutput")

    with tile.TileContext(nc) as tc:
        with (
            tc.tile_pool(name="singles", bufs=1) as singles,
            tc.tile_pool(name="weights", bufs=1) as wpool,
            tc.tile_pool(name="dq", bufs=2) as dqpool,
            tc.tile_pool(name="scexp", bufs=2) as scpool,
            tc.tile_pool(name="xt", bufs=2) as xtpool,
            tc.tile_pool(name="yout", bufs=3) as ypool,
            tc.tile_pool(name="psum_y", bufs=2, space="PSUM") as psum_y,
        ):
            bias_sb = singles.tile([P, OUT_SH], F32)
            nc.gpsimd.dma_start(out=bias_sb, in_=bi[:, :].to_broadcast((P, OUT_SH)))

            # x pipeline: x arrives bf16 (host-cast); per (chunk, k-tile) a
            # big xbar transpose DRAM->SBUF: [CHUNK tok, 128 k] -> [128, CHUNK]
            xts = {}

            def load_chunk(mc):
                rows = x[mc * CHUNK : (mc + 1) * CHUNK, :]
                for kt in range(n_kt):
                    xt = xtpool.tile([P, CHUNK], BF16, tag=f"xt{kt}")
                    nc.sync.dma_start_transpose(
                        xt, rows[:, kt * P : (kt + 1) * P]
                    )
                    xts[(mc, kt)] = xt

            load_chunk(0)

            # prefetch weight shard DMAs so dequant starts ASAP
            qw_tiles = []
            for t in range(n_t):
                qw_t = dqpool.tile([P, OUT_SH], I32, tag="qw")
                nc.scalar.dma_start(qw_t, qw[t * P : (t + 1) * P, :])
                qw_tiles.append(qw_t)

            if n_chunks > 1:
                load_chunk(1)

            # ---- dequantize weight shard into 32 resident tiles ----
            # w[kk, n] = sc_exp[kk,n] * nib_j(qw)[kk,n] - sc_exp[kk,n]*(zq[kk,n]+1)
            w_tiles = []
            for t in range(n_t):
                # scale_exp[kk, n] = scales[8t + kk//16, n]
                scale_exp = scpool.tile([P, OUT_SH], F32, tag="scale_exp")
                nc.gpsimd.dma_start(
                    out=scale_exp,
                    in_=bass.AP(
                        tensor=sc,
                        offset=t * 8 * OUT_SH,
                        ap=[[OUT_SH, 8], [0, 16], [1, OUT_SH]],
                    ),
                )
                # qz_exp[kk, m] = qzeros[8t + kk//16, m]
                qz_exp = scpool.tile([P, OUT_SH // 8], I32, tag="qz_exp")
                nc.gpsimd.dma_start(
                    out=qz_exp,
                    in_=bass.AP(
                        tensor=qz,
                        offset=t * 8 * (OUT_SH // 8),
                        ap=[[OUT_SH // 8, 8], [0, 16], [1, OUT_SH // 8]],
                    ),
                )
                # szp_exp[kk, m*8+j] = scale_exp[kk, m*8+j] * (zq_nib_j[kk, m]+1)
                szp_exp = scpool.tile([P, OUT_SH], BF16, tag="szp_exp")
                szp_r = szp_exp.rearrange("p (m j) -> p m j", j=8)
                sc_r = scale_exp.rearrange("p (m j) -> p m j", j=8)
                zq_nib = scpool.tile([P, OUT_SH // 8], I32, tag="zq_nib")
                for j in range(8):
                    nc.vector.tensor_scalar(
                        out=zq_nib,
                        in0=qz_exp,
                        scalar1=4 * j,
                        scalar2=0xF,
                        op0=ALU.logical_shift_right,
                        op1=ALU.bitwise_and,
                    )
                    nc.vector.scalar_tensor_tensor(
                        out=szp_r[:, :, j],
                        in0=zq_nib,
                        scalar=1.0,
                        in1=sc_r[:, :, j],
                        op0=ALU.add,
                        op1=ALU.mult,
                    )
                qw_t = qw_tiles[t]
                for j in range(8):
                    kt = t * 8 + j
                    nib = dqpool.tile([P, OUT_SH], I32, tag="nib")
                    nc.vector.tensor_scalar(
                        out=nib,
                        in0=qw_t,
                        scalar1=4 * j,
                        scalar2=0xF,
                        op0=ALU.logical_shift_right,
                        op1=ALU.bitwise_and,
                    )
                    w = wpool.tile([P, OUT_SH], BF16, tag=f"w{kt}")
                    nc.vector.tensor_tensor(
                        out=w, in0=nib, in1=scale_exp, op=ALU.mult
                    )
                    nc.vector.tensor_sub(w, w, szp_exp)
                    w_tiles.append(w)

            # ---- main loop: chunks of 4 m-tiles, processed in halves of 2
            # (2 y-psum tiles x 2 pool bufs = all 8 PSUM banks) ----
            for mc in range(n_chunks):
                if mc + 2 < n_chunks:
                    load_chunk(mc + 2)
                for half in range(B // 2):
                    mis = (2 * half, 2 * half + 1)
                    ypsums = {}
                    for mi in mis:
                        yp = psum_y.tile([P, OUT_SH], F32, tag="y")
                        ypsums[mi] = yp
                    for kt in range(n_kt):
                        for mi in mis:
                            lhsT = xts[(mc, kt)][:, mi * P : (mi + 1) * P]
                            for h in range(2):
                                nc.tensor.matmul(
                                    ypsums[mi][:, h * 512 : (h + 1) * 512],
                                    lhsT=lhsT,
                                    rhs=w_tiles[kt][:, h * 512 : (h + 1) * 512],
                                    start=(kt == 0),
                                    stop=(kt == n_kt - 1),
                                )
                    for mi in mis:
                        y_sb = ypool.tile([P, OUT_SH], F32, tag="y_sb")
                        nc.vector.tensor_add(y_sb, ypsums[mi], bias_sb)
                        nc.scalar.dma_start(
                            out[
                                mc * CHUNK + mi * P : mc * CHUNK + (mi + 1) * P, :
                            ],
                            y_sb,
                        )
                for key in [k for k in xts if k[0] == mc]:
                    del xts[key]

    nc.compile()
    return nc


_NC_CACHE = {}


def _get_nc(tok=TOK_SH):
    if tok not in _NC_CACHE:
        _NC_CACHE[tok] = build_nc(tok)
    return _NC_CACHE[tok]


# Device k-tile kt = t*8 + j holds W rows k = 8*(t*128+i) + j (nibble-major
# unpack order). Permute x columns so contiguous 128-col block kt matches.
_C = np.arange(IN_F)
_KT, _I = divmod(_C, P)
_T, _J = divmod(_KT, 8)
K_PERM = _T * 1024 + 8 * _I + _J


def _shard_inputs(x, qweight, qzeros, scales, bias, tok_sh=TOK_SH):
    import ml_dtypes

    xp = np.ascontiguousarray(x[:, K_PERM]).astype(ml_dtypes.bfloat16)
    in_maps = []
    for c in range(N_CORES):
        ti, oj = divmod(c, N_OUT_SHARDS)
        sl = slice(oj * OUT_SH, (oj + 1) * OUT_SH)
        slz = slice(oj * (OUT_SH // 8), (oj + 1) * (OUT_SH // 8))
        in_maps.append(
            {
                "x": np.ascontiguousarray(xp[ti * tok_sh : (ti + 1) * tok_sh]),
                "qw": np.ascontiguousarray(qweight[:, sl], dtype=np.int32),
                "qz": np.ascontiguousarray(qzeros[:, slz], dtype=np.int32),
                "sc": np.ascontiguousarray(scales[:, sl], dtype=np.float32),
                "bi": np.ascontiguousarray(
                    bias[sl].reshape(1, OUT_SH), dtype=np.float32
                ),
            }
        )
    return in_maps


def _assemble(per_core, tok_sh=TOK_SH):
    out = np.empty((N_TOK_SHARDS * tok_sh, OUT_F), dtype=np.float32)
    for c in range(N_CORES):
        ti, oj = divmod(c, N_OUT_SHARDS)
        out[ti * tok_sh : (ti + 1) * tok_sh, oj * OUT_SH : (oj + 1) * OUT_SH] = (
            per_core[c]["out"]
        )
    return out


class PjrtRunner:
    """Builds the shard_map'd bass executable once; supports timed re-runs."""

    def __init__(self, nc):
        import jax
        from jax.sharding import Mesh, PartitionSpec
        from jax.experimental.shard_map import shard_map
        from concourse import bass2jax, mybir as mb

        self.jax = jax
        bass2jax.install_neuronx_cc_hook()

        partition_name = (
            nc.partition_id_tensor.name if nc.partition_id_tensor else None
        )
        in_names, out_names, out_avals, zero_outs = [], [], [], []
        for alloc in nc.m.functions[0].allocations:
            if not isinstance(alloc, mb.MemoryLocationSet):
                continue
            name = alloc.memorylocations[0].name
            if alloc.kind == "ExternalInput":
                if name != partition_name:
                    in_names.append(name)
            elif alloc.kind == "ExternalOutput":
                shape = tuple(alloc.tensor_shape)
                dtype = mb.dt.np(alloc.dtype)
                out_names.append(name)
                out_avals.append(jax.core.ShapedArray(shape, dtype))
                zero_outs.append(np.zeros(shape, dtype))
        self.in_names = in_names
        self.out_names = out_names
        self.zero_outs = zero_outs
        n_params = len(in_names)
        all_in_names = in_names + out_names
        if partition_name is not None:
            all_in_names.append(partition_name)

        def _body(*args):
            operands = list(args)
            if partition_name is not None:
                operands.append(bass2jax.partition_id_tensor())
            outs = bass2jax._bass_exec_p.bind(
                *operands,
                out_avals=tuple(out_avals),
                in_names=tuple(all_in_names),
                out_names=tuple(out_names),
                lowering_input_output_aliases=(),
                sim_require_finite=True,
                sim_require_nnan=True,
                nc=nc,
            )
            return tuple(outs)

        devices = jax.devices()[:N_CORES]
        self.mesh = Mesh(np.asarray(devices), ("core",))
        in_specs = (PartitionSpec("core"),) * (n_params + len(out_names))
        out_specs = (PartitionSpec("core"),) * len(out_names)
        # no donation: lets us re-run with the same device-resident inputs
        self.fn = jax.jit(
            shard_map(
                _body,
                mesh=self.mesh,
                in_specs=in_specs,
                out_specs=out_specs,
                check_rep=False,
            ),
            keep_unused=True,
        )
        self.out_avals = out_avals

    def stage_inputs(self, in_maps):
        import jax
        from jax.sharding import NamedSharding, PartitionSpec

        sharding = NamedSharding(self.mesh, PartitionSpec("core"))
        args = []
        for name in self.in_names:
            concat = np.concatenate([np.asarray(m[name]) for m in in_maps], axis=0)
            args.append(jax.device_put(concat, sharding))
        for z in self.zero_outs:
            zc = np.zeros((N_CORES * z.shape[0], *z.shape[1:]), z.dtype)
            args.append(jax.device_put(zc, sharding))
        self.args = args

    def run(self):
        outs = self.fn(*self.args)
        self.jax.block_until_ready(outs)
        return outs

    def outputs_to_numpy(self, outs):
        per_core = []
        for c in range(N_CORES):
            per_core.append(
                {
                    name: np.asarray(outs[i]).reshape(
                        N_CORES, *self.out_avals[i].shape
                    )[c]
                    for i, name in enumerate(self.out_names)
                }
            )
        return per_core


_RUNNER_CACHE = {}


def get_runner(tok=TOK_SH):
    if tok not in _RUNNER_CACHE:
        _RUNNER_CACHE[tok] = PjrtRunner(_get_nc(tok))
    return _RUNNER_CACHE[tok]


def _kernel_np_fallback(x, qweight, qzeros, scales, g_idx, bias):
    shifts = (np.arange(8, dtype=np.int64) * 4)[None, :, None]
    wq = ((qweight.astype(np.int64)[:, None, :] >> shifts) & 0xF).reshape(
        IN_F, qweight.shape[1]
    )
    zq = (
        (qzeros.astype(np.int64)[:, :, None] >> shifts.reshape(1, 1, 8)) & 0xF
    ).reshape(qzeros.shape[0], -1) + 1
    w = scales[g_idx] * (wq.astype(np.float32) - zq[g_idx].astype(np.float32))
    return (x.astype(np.float32) @ w + bias).astype(np.float32)


def kernel(x, qweight, qzeros, scales, g_idx, bias):
    x = np.asarray(x)
    qweight = np.asarray(qweight)
    qzeros = np.asarray(qzeros)
    scales = np.asarray(scales)
    g_idx = np.asarray(g_idx)
    bias = np.asarray(bias)

    if not np.array_equal(
        g_idx, (np.arange(IN_F, dtype=np.int64) // GROUPSIZE).astype(g_idx.dtype)
    ):
        return _kernel_np_fallback(x, qweight, qzeros, scales, g_idx, bias)

    runner = get_runner()
    runner.stage_inputs(_shard_inputs(x, qweight, qzeros, scales, bias))
    outs = runner.run()
    return _assemble(runner.outputs_to_numpy(outs))
